# revision 31
# baseline (speedup 1.0000x reference)
"""Butterfly-Conv2d (nn_BConv2d) Trainium2 kernel, v3.

Math (reference): x(B=64,IC=16,32,32) -> y=x.reshape(IC,B,N=1024)[:,:,bitrev];
broadcast over OC=32; 10 radix-2 butterfly layers with per-(ic,oc) twiddles;
mean over ic; + bias -> (B,OC,32,32).

Strategy (v3):
  * Shard over OC: 8 cores x 4 oc each; every core holds all 16 ic so the
    ic-mean is core-local (no collective). Host concatenates oc slices.
  * Stage A (PE, bf16): butterfly layers 0..6 compose on host into 8
    chunk-local 128x128 blocks; LAYER 7 IS FOLDED IN as a diagonal scale of
    the block output rows -> 16 scaled stationaries per (ic,oc). Each output
    chunk c' accumulates two matmuls (q=0,1) in PSUM. Device layout
    y[ic] = [128, 512]: partition p = n & 127, free = c*64 + b, c = n >> 7.
  * Stage B (vector engines, fp32 intermediates -- this DVE runs fp32-input
    ops at ~2x the rate of bf16-input ones): layers 8 and 9 as wide [128,*]
    multiplies with chunk-permuted broadcast-coefficient access patterns.
    Work is split DVE / GpSimd / Act (Act does per-chunk scale-copies, the
    only form it supports).
  * ic-accumulation on the PE: the two layer-9 product tiles (bf16) are
    accumulated onto a per-oc PSUM bank via identity-stationary matmuls,
    eliminating all reduction adds from the vector engines. 1/IC is folded
    into layer-9 coeffs, bias is added during the final PSUM drain.
"""

import numpy as np
import ml_dtypes

B, IC, OC, H, W = 64, 16, 32, 32, 32
N = H * W          # 1024
NCORES = 8
OCL = OC // NCORES  # 4 oc per core
NCH = 8            # free-dim chunks (n9n8n7)
P = 128            # partitions (n6..n0)
SB = 128           # composed stage-A block size (layers 0..6)
NBLK = N // SB     # 8 blocks per (ic,oc)
NC89 = 32          # stage-B coeffs per (ic,oc): 2 layers x 2 q x 8 chunks

BF16 = ml_dtypes.bfloat16


def _bitrev(n):
    bits = int(np.log2(n))
    idx = np.arange(n, dtype=np.int64)
    rev = np.zeros(n, dtype=np.int64)
    for b in range(bits):
        rev = (rev << 1) | ((idx >> b) & 1)
    return rev


def _compose_stageA(tw):
    """Compose butterfly layers 0..6 into A[ic,oc,8,128,128] (f32)."""
    ic, oc = tw.shape[0], tw.shape[1]
    A = np.zeros((ic, oc, NBLK, SB, SB), dtype=np.float32)
    A[:] = np.eye(SB, dtype=np.float32)
    for l in range(7):
        s = 1 << l
        nb_loc = SB // (2 * s)
        t = tw[:, :, l].reshape(ic, oc, N // (2 * s), s, 2, 2)
        t = t.reshape(ic, oc, NBLK, nb_loc, s, 2, 2)
        Av = A.reshape(ic, oc, NBLK, nb_loc, 2, s, SB)
        a0 = Av[:, :, :, :, 0].copy()
        a1 = Av[:, :, :, :, 1].copy()
        Av[:, :, :, :, 0] = t[..., 0, 0, None] * a0 + t[..., 0, 1, None] * a1
        Av[:, :, :, :, 1] = t[..., 1, 0, None] * a0 + t[..., 1, 1, None] * a1
    return A


def _fold_layer7(A, tw):
    """Fold layer 7 into stage A: W'[c',q] = diag(t7_q[c']) @ A[src7(c',q)].

    Layer 7 (s=128): n = k*256 + q*128 + p; out chunk c' = (k, p_out);
    src chunk = 2k + q; coeff t7[k, p, p_out, q] (diag over p = out row).
    Returns W[ic, oc, 8(c'), 2(q), 128, 128] f32 and src7[c', q] ints.
    """
    ic, oc = tw.shape[0], tw.shape[1]
    t7 = tw[:, :, 7].reshape(ic, oc, 4, 128, 2, 2)  # [k, p, p_out, q]
    W = np.zeros((ic, oc, NCH, 2, SB, SB), dtype=np.float32)
    src7 = np.zeros((NCH, 2), dtype=np.int64)
    for cp in range(NCH):
        k, p_out = cp >> 1, cp & 1
        for q in range(2):
            s = 2 * k + q
            src7[cp, q] = s
            W[:, :, cp, q] = t7[:, :, k, :, p_out, q, None] * A[:, :, s]
    return W, src7


def _stageB_coeffs(tw):
    """Coeff vectors for layers 8, 9: ct[ic, oc, 128, (l,q,c')=32] f32.

      layer 8 (s=256): n = k*512 + q*256 + j, j=(n7,p); out c' = (k,p_out,n7);
        src = 4k + 2q + n7; coeff t8[k, n7*128+p, p_out, q].
      layer 9 (s=512): n = q*512 + j, j=(n8,n7,p); out c' = (p_out,n8,n7);
        src = 4q + 2n8 + n7; coeff t9[0, (n8n7)*128+p, p_out, q] / IC.
    """
    ic, oc = tw.shape[0], tw.shape[1]
    t8 = tw[:, :, 8].reshape(ic, oc, 2, 256, 2, 2)
    t9 = tw[:, :, 9].reshape(ic, oc, 1, 512, 2, 2)
    ct = np.zeros((ic, oc, P, 2, 2, NCH), dtype=np.float32)
    src = np.zeros((2, NCH, 2), dtype=np.int64)
    pr = np.arange(P)
    for cp in range(NCH):
        k8, p_out8, n7_8 = cp >> 2, (cp >> 1) & 1, cp & 1
        p_out9 = cp >> 2
        for q in range(2):
            ct[:, :, :, 0, q, cp] = t8[:, :, k8, n7_8 * 128 + pr, p_out8, q]
            src[0, cp, q] = 4 * k8 + 2 * q + n7_8
            ct[:, :, :, 1, q, cp] = (
                t9[:, :, 0, (cp & 3) * 128 + pr, p_out9, q] / IC
            )
            src[1, cp, q] = 4 * q + (cp & 3)
    return ct.reshape(ic, oc, P, NC89), src


def _prep_host(x, twiddle, bias):
    perm = _bitrev(N)
    y = np.ascontiguousarray(x).reshape(IC, B, N)[:, :, perm]
    y_dev = np.ascontiguousarray(
        y.reshape(IC, B, NCH, P).transpose(0, 3, 2, 1)
    ).reshape(IC, P, NCH * B).astype(BF16)

    tw = np.asarray(twiddle, dtype=np.float32)
    A = _compose_stageA(tw)
    W, src7 = _fold_layer7(A, tw)
    ct, src = _stageB_coeffs(tw)

    bias_np = np.asarray(bias, dtype=np.float32).reshape(OC, NCH, P)
    ident = np.eye(P, dtype=np.float32)

    in_maps = []
    for core in range(NCORES):
        osl = slice(core * OCL, (core + 1) * OCL)
        # lhsT: w[ic,o][p_k, (c'*2+q)*128 + m] = W[ic,o,c',q][m, p_k]
        w = np.ascontiguousarray(
            W[:, osl].transpose(0, 1, 5, 2, 3, 4)  # [ic,o,k,c',q,m]
        ).astype(BF16)
        ctc = np.ascontiguousarray(
            ct[:, osl].transpose(2, 0, 1, 3)  # [p, ic, o, 32]
        ).reshape(P, IC * OCL * NC89).astype(BF16)
        # fp32 copy of the layer-8 q=1 coeffs for ActE narrow ops
        ct32 = np.ascontiguousarray(
            ct[:, osl, :, NCH : 2 * NCH].transpose(2, 0, 1, 3)  # [p, ic, o, 8]
        ).reshape(P, IC * OCL * NCH).astype(np.float32)
        bc = np.ascontiguousarray(
            bias_np[osl].transpose(2, 0, 1)  # [p, o, c]
        ).reshape(P, OCL * NCH).astype(np.float32)
        in_maps.append(
            {
                "y": y_dev,
                "w": w.reshape(IC, OCL, P, NCH * 2 * SB),
                "ct": ctc,
                "ct32": ct32,
                "bias": bc,
                "ident": ident,
                "identb": ident.astype(BF16),
            }
        )
    return in_maps, src7, src


def _emulate_core(im, src7, src):
    y = im["y"].astype(np.float32).reshape(IC, P, NCH, B)
    w = im["w"].astype(np.float32).reshape(IC, OCL, P, NCH, 2, SB)
    ct = im["ct"].astype(np.float32).reshape(P, IC, OCL, 2, 2, NCH)
    bias = im["bias"].astype(np.float32).reshape(P, OCL, NCH)
    out = np.zeros((OCL, P, NCH, B), dtype=np.float32)
    for o in range(OCL):
        for ic in range(IC):
            z = np.zeros((P, NCH, B), dtype=np.float32)
            for cp in range(NCH):
                for q in range(2):
                    lhsT = w[ic, o, :, cp, q]  # [k, m]
                    z[:, cp] += lhsT.T @ y[ic, :, int(src7[cp, q])]
            cur = z
            for l in range(2):
                nxt = np.zeros_like(cur)
                for cp in range(NCH):
                    s0, s1 = int(src[l, cp, 0]), int(src[l, cp, 1])
                    t0 = ct[:, ic, o, l, 0, cp, None] * cur[:, s0]
                    t1 = ct[:, ic, o, l, 1, cp, None] * cur[:, s1]
                    if l == 0:
                        t1 = t1.astype(BF16).astype(np.float32)
                    else:
                        t0 = t0.astype(BF16).astype(np.float32)
                    nxt[:, cp] = t0 + t1
                cur = nxt
            out[o] += cur
        out[o] += bias[:, o, :, None]
    return out.reshape(OCL, P, NCH * B)


def _bcast_coef(ct_ap, ic, o, l, q):
    off = (ic * OCL + o) * NC89 + (l * 2 + q) * NCH
    sl = ct_ap[:, off : off + NCH]
    return sl.unsqueeze(2).broadcast_to([P, NCH, B])


def _perm8(z_ap, q):
    # out c' = (k:2, p_out:2, j7:2); src = 4k + 2q + j7
    r = z_ap.rearrange("p (k q j b) -> p k q j b", k=2, q=2, j=2, b=B)[:, :, q]
    return r.unsqueeze(2).broadcast_to([P, 2, 2, 2, B])


def _perm9(z_ap, q):
    # out c' = (p_out:2, n8:2, n7:2); src = 4q + 2n8 + n7
    r = z_ap.rearrange("p (q j b) -> p q j b", q=2, j=4, b=B)[:, q]
    return r.unsqueeze(1).broadcast_to([P, 2, 4, B])


def _emit_acc(nc, id_t, idb_t, pending):
    acc, t9a, t9b, ic = pending
    nc.tensor.matmul(
        acc[:], idb_t[:], t9a[:],
        start=(ic == 0), stop=False, skip_group_check=True,
    )
    nc.tensor.matmul(
        acc[:], id_t[:], t9b[:],
        start=False, stop=(ic == IC - 1), skip_group_check=True,
    )


def _build_program(src7, src):
    import concourse.bacc as bacc
    import concourse.mybir as mybir
    from concourse.tile import TileContext

    bf16 = mybir.dt.bfloat16
    f32 = mybir.dt.float32
    MULT, ADD = mybir.AluOpType.mult, mybir.AluOpType.add

    nc = bacc.Bacc(None, target_bir_lowering=False)
    y_d = nc.dram_tensor("y", (IC, P, NCH * B), bf16, kind="ExternalInput")
    w_d = nc.dram_tensor(
        "w", (IC, OCL, P, NCH * 2 * SB), bf16, kind="ExternalInput"
    )
    ct_d = nc.dram_tensor("ct", (P, IC * OCL * NC89), bf16, kind="ExternalInput")
    ct32_d = nc.dram_tensor("ct32", (P, IC * OCL * NCH), f32, kind="ExternalInput")
    bias_d = nc.dram_tensor("bias", (P, OCL * NCH), f32, kind="ExternalInput")
    id_d = nc.dram_tensor("ident", (P, P), mybir.dt.float32r, kind="ExternalInput")
    idb_d = nc.dram_tensor("identb", (P, P), bf16, kind="ExternalInput")
    o_d = nc.dram_tensor("o", (OCL, P, NCH * B), f32, kind="ExternalOutput")

    with TileContext(nc) as tc:
        with (
            tc.tile_pool(name="const", bufs=1) as cpool,
            tc.tile_pool(name="ypool", bufs=1) as ypool,
            tc.tile_pool(name="wpool", bufs=8) as wpool,
            tc.tile_pool(name="tmp", bufs=12) as tpool,
            tc.tile_pool(name="t9p", bufs=10) as t9pool,
            tc.tile_pool(name="outp", bufs=2) as opool,
            tc.tile_pool(name="psum", bufs=4, space="PSUM") as pspool,
            tc.tile_pool(name="psacc", bufs=OCL, space="PSUM") as papool,
        ):
            ytile = ypool.tile([P, IC * NCH * B], bf16, tag="y")
            for ic in range(IC):
                nc.sync.dma_start(
                    out=ytile[:, ic * NCH * B : (ic + 1) * NCH * B], in_=y_d[ic]
                )
            ct_t = cpool.tile([P, IC * OCL * NC89], bf16, tag="ct")
            nc.sync.dma_start(out=ct_t[:], in_=ct_d[:, :])
            ct32_t = cpool.tile([P, IC * OCL * NCH], f32, tag="ct32")
            nc.sync.dma_start(out=ct32_t[:], in_=ct32_d[:, :])
            bias_t = cpool.tile([P, OCL * NCH], f32, tag="bias")
            nc.sync.dma_start(out=bias_t[:], in_=bias_d[:, :])
            id_t = cpool.tile([P, P], mybir.dt.float32r, tag="ident")
            nc.sync.dma_start(out=id_t[:], in_=id_d[:, :])
            idb_t = cpool.tile([P, P], bf16, tag="identb")
            nc.sync.dma_start(out=idb_t[:], in_=idb_d[:, :])

            accs = [
                papool.tile([P, NCH * B], f32, tag="acc", name=f"acc{o}")
                for o in range(OCL)
            ]
            pending = []
            for ic in range(IC):
                for o in range(OCL):
                    acc = accs[o]
                    wtile = wpool.tile([P, NCH * 2 * SB], bf16)
                    nc.sync.dma_start(out=wtile[:], in_=w_d[ic, o])
                    z = pspool.tile([P, NCH * B], f32)
                    # stage A + layer 7: two accumulated matmuls per chunk
                    for cp in range(NCH):
                        for q in range(2):
                            wi = (cp * 2 + q) * SB
                            s = int(src7[cp, q])
                            nc.tensor.matmul(
                                z[:, cp * B : (cp + 1) * B],
                                wtile[:, wi : wi + SB],
                                ytile[:, (ic * NCH + s) * B : (ic * NCH + s + 1) * B],
                                start=(q == 0),
                                stop=(q == 1),
                            )
                    while len(pending) >= 6:
                        _emit_acc(nc, id_t, idb_t, pending.pop(0))
                    zap = z[:]
                    cb32 = (ic * OCL + o) * NCH
                    # layer 8: q=0 as one wide DVE multiply (fp32 out);
                    # q=1 split Act narrow scale-copies (chunks 0..5, bf16)
                    # + one [P,128] DVE op (chunks 6,7).
                    t8a = tpool.tile([P, NCH * B], f32)
                    t8b = tpool.tile([P, NCH * B], bf16, tag="t8b")
                    y8 = tpool.tile([P, NCH * B], f32)
                    nc.vector.tensor_tensor(
                        t8a[:], _perm8(zap, 0), _bcast_coef(ct_t[:], ic, o, 0, 0), MULT
                    )
                    for cp in range(6):
                        s1 = int(src[0, cp, 1])
                        nc.scalar.activation(
                            t8b[:, cp * B : (cp + 1) * B],
                            zap[:, s1 * B : (s1 + 1) * B],
                            mybir.ActivationFunctionType.Copy,
                            scale=ct32_t[:, cb32 + cp : cb32 + cp + 1],
                        )
                    # chunks 6,7 of q=1: c'=(k=1,p_out=1,j7), src=6+j7 adjacent
                    cof = ct_t[
                        :,
                        (ic * OCL + o) * NC89 + NCH + 6 : (ic * OCL + o) * NC89
                        + NCH
                        + 8,
                    ]
                    cof = cof.unsqueeze(2).broadcast_to([P, 2, B])
                    nc.vector.tensor_tensor(
                        t8b[:, 6 * B : 8 * B].rearrange("p (c b) -> p c b", c=2),
                        zap[:, 6 * B : 8 * B].rearrange("p (c b) -> p c b", c=2),
                        cof,
                        MULT,
                    )
                    if ic % 2 == 1:
                        nc.gpsimd.tensor_add(y8[:], t8a[:], t8b[:])
                    else:
                        nc.vector.tensor_add(y8[:], t8a[:], t8b[:])
                    # layer 9: GpSimd (q=0) + DVE (q=1) multiplies -> fp32
                    t9a = t9pool.tile([P, NCH * B], bf16)
                    t9b = t9pool.tile([P, NCH * B], mybir.dt.float32r)
                    nc.gpsimd.tensor_tensor(
                        t9a[:], _perm9(y8[:], 0), _bcast_coef(ct_t[:], ic, o, 1, 0), MULT
                    )
                    nc.vector.tensor_tensor(
                        t9b[:], _perm9(y8[:], 1), _bcast_coef(ct_t[:], ic, o, 1, 1), MULT
                    )
                    # ic-accumulation on PE (deferred three pairs so the
                    # id-MMs never head-of-line-block stage matmuls in the
                    # PE queue while waiting on vector-engine products)
                    pending.append((acc, t9a, t9b, ic))
            while pending:
                _emit_acc(nc, id_t, idb_t, pending.pop(0))
            for o in range(OCL):
                out_t = opool.tile([P, NCH * B], f32)
                acc = accs[o]
                bias_ap = (
                    bias_t[:, o * NCH : (o + 1) * NCH]
                    .unsqueeze(2)
                    .broadcast_to([P, NCH, B])
                )
                nc.vector.tensor_tensor(out_t[:], acc[:], bias_ap, ADD)
                nc.sync.dma_start(out=o_d[o], in_=out_t[:])
    nc.finalize()
    return nc


_LAST_RESULTS = {"exec_time_ns": None}


def kernel(x, twiddle, bias, _trace=False, _emulate=False):
    in_maps, src7, src = _prep_host(
        np.asarray(x), np.asarray(twiddle), np.asarray(bias)
    )
    if _emulate:
        outs = [_emulate_core(im, src7, src) for im in in_maps]
    else:
        from concourse.bass_utils import run_bass_kernel_spmd

        nc = _build_program(src7, src)
        res = run_bass_kernel_spmd(
            nc, in_maps, list(range(NCORES)), trace=_trace
        )
        _LAST_RESULTS["exec_time_ns"] = res.exec_time_ns
        _LAST_RESULTS["mean_exec_time_ns"] = res.mean_exec_time_ns
        outs = [r["o"] for r in res.results]
    full = np.concatenate(
        [
            np.asarray(o, dtype=np.float32)
            .reshape(OCL, P, NCH, B)
            .transpose(0, 3, 2, 1)
            .reshape(OCL, B, N)
            for o in outs
        ],
        axis=0,
    )
    return np.ascontiguousarray(full).reshape(B, OC, H, W).astype(np.float32)
